# revision 1
# baseline (speedup 1.0000x reference)
"""CRF loss kernel for Trainium2 (8 NeuronCores, data-parallel over batch).

reference: mean_b( logZ_b - score_b ) for a linear-chain CRF with
B=256, S=512, T=128.

Denominator (logZ, 99.9% of the FLOPs) runs on device in exp space:
    u_0[t, b] = exp(start[t]) * exp(em[b, 0, t])
    u_s       = (A^T u_{s-1}) o exp(em_s - kappa)    A = exp(transitions)
    logZ_b    = log( sum_t u_S[t,b] * exp(end[t]) ) + (S-1) * kappa
kappa is the exact per-step log-mass growth of batch 0, computed on the
host with one fp64 log-space forward (~3 ms) and pre-subtracted from the
emissions, so u stays O(1) (per-batch drift is a +-10 random walk
against an fp32 budget of +-87) and the device needs NO runtime
renormalization — every scan step is exactly one bf16 matmul (fp32 PSUM)
plus one fused DVE multiply. Validated to ~3e-5 relative error against
the fp32 reference.

Layout per core: state vectors are [T=128 partitions, batch free]. Each
scan step is chain-latency bound (~430-460 ns: matmul drain + DVE
PSUM-access + two semaphore hops), so the serial depth is halved by
meeting in the middle: logZ is the bilinear form x^T (prod_i A diag(e_i)) u_0,
computed as alpha (forward from step 0, u_i = (A^T u_{i-1}) o e_i) and
beta (backward from step 511, beta_{i-1} = A (e_i o beta_i), stationary
exp(transitions)^T) running concurrently as two independent
TensorE<->VectorE chains that interleave on the engines; 256 rounds
instead of 511. Z = sum_t u_255[t] * beta_255[t].

Numerator (score of the tagged path) is a handful of gathers summing to
~0.1% of the FLOPs; it is computed on the host in fp64.
"""

import numpy as np
import ml_dtypes

B, S, T = 256, 512, 128
NCORES = 8
BC = B // NCORES          # 32 batches per core
MEET = 255                # forward computes u_MEET, backward beta_MEET
CH = 64                   # emission chunk length (steps per DMA)

_nc_cache = None
LAST_RESULTS = None       # BassKernelResults of the most recent device run


def _build_nc():
    import concourse.bacc as bacc
    import concourse.mybir as mybir
    import concourse.tile as tile

    fp32 = mybir.dt.float32
    bf16 = mybir.dt.bfloat16
    Exp = mybir.ActivationFunctionType.Exp
    Ln = mybir.ActivationFunctionType.Ln
    mult = mybir.AluOpType.mult
    add = mybir.AluOpType.add

    nc = bacc.Bacc("TRN2", target_bir_lowering=False, debug=False)

    em_t = nc.dram_tensor("em_t", [T, S, BC], bf16, kind="ExternalInput")
    # packed constants: [exp(trans) | exp(trans).T]
    cpack = nc.dram_tensor("cpack", [T, 2 * T], bf16, kind="ExternalInput")
    se_exp = nc.dram_tensor("se_exp", [T, 2], fp32, kind="ExternalInput")
    denom = nc.dram_tensor("denom", [1, BC], fp32, kind="ExternalOutput")

    # Lead-in chunks at BOTH ends are small so their exp clears ACT quickly
    # and both scans start early; each direction consumes 64 steps per
    # ~28 us while a chunk DMA+exp takes ~3 us, so neither ever starves.
    chunks = ([(0, 1), (1, 15), (16, 48)]
              + [(s, CH) for s in range(CH, S - CH, CH)]
              + [(448, 48), (496, 15), (511, 1)])
    # DMA/exp emission order: both ends first, then inward
    order = [0, len(chunks) - 1, 1, len(chunks) - 2, 2, len(chunks) - 3]
    mid = [i for i in range(len(chunks)) if i not in order]
    order += [mid[k // 2] if k % 2 == 0 else mid[-1 - k // 2]
              for k in range(len(mid))]

    with tile.TileContext(nc) as tc:
        with (
            tc.tile_pool(name="const", bufs=1) as constp,
            tc.tile_pool(name="emraw", bufs=4) as emraw_p,
            # all exp(em) chunks stay resident (~4 MB of SBUF)
            tc.tile_pool(name="emexp", bufs=len(chunks)) as emexp_p,
            tc.tile_pool(name="uf", bufs=2) as ufp,
            tc.tile_pool(name="wb", bufs=2) as wbp,
            tc.tile_pool(name="vps", bufs=2, space="PSUM") as vp,
            tc.tile_pool(name="bps", bufs=2, space="PSUM") as bp,
            tc.tile_pool(name="side", bufs=2) as sidep,
        ):
            emexp_tiles = {}

            def load_chunk(ci):
                s0, ln = chunks[ci]
                raw = emraw_p.tile([T, ln, BC], bf16, tag="emraw")
                nc.sync.dma_start(raw[:], em_t[:, s0:s0 + ln, :])
                ex = emexp_p.tile([T, ln, BC], bf16, tag="emexp")
                nc.scalar.activation(ex[:], raw[:], Exp)
                emexp_tiles[ci] = ex

            def em_slice(s):
                for ci, (s0, ln) in enumerate(chunks):
                    if s0 <= s < s0 + ln:
                        return emexp_tiles[ci][:, s - s0, :]
                raise AssertionError(s)

            load_chunk(order[0])
            load_chunk(order[1])

            se_tile = constp.tile([T, 2], fp32)
            nc.sync.dma_start(se_tile[:], se_exp[:])
            cp_tile = constp.tile([T, 2 * T], bf16)
            nc.sync.dma_start(cp_tile[:], cpack[:])
            a_tile = cp_tile[:, 0:T]
            at_tile = cp_tile[:, T:2 * T]
            sexp_ap = se_tile[:, 0:1]
            eexp_ap = se_tile[:, 1:2]
            ones_t = constp.tile([T, 1], bf16)
            nc.gpsimd.memset(ones_t[:], 1.0)

            for ci in order[2:]:
                load_chunk(ci)

            # forward init: u_0 = exp(em_0) * exp(start)
            u = ufp.tile([T, BC], bf16)
            nc.vector.tensor_scalar(u[:], em_slice(0), sexp_ap, None, mult)
            # backward init: w_511 = exp(em_511) * exp(end)
            w = wbp.tile([T, BC], bf16)
            nc.vector.tensor_scalar(w[:], em_slice(S - 1), eexp_ap, None, mult)

            beta_ps = None
            for r in range(1, S - MEET):
                # forward step s = r (runs for r <= MEET)
                if r <= MEET:
                    v = vp.tile([T, BC], fp32, tag="vf")
                    nc.tensor.matmul(v[:], a_tile, u[:],
                                     start=True, stop=True)
                    u_new = ufp.tile([T, BC], bf16)
                    nc.vector.tensor_tensor(u_new[:], v[:], em_slice(r), mult)
                    u = u_new
                # backward step i = S - r: beta_{i-1} = A (e_i o beta_i)
                i = S - r
                beta_ps = bp.tile([T, BC], fp32, tag="vb")
                nc.tensor.matmul(beta_ps[:], at_tile, w[:],
                                 start=True, stop=True)
                if i - 1 > MEET:
                    w_new = wbp.tile([T, BC], bf16)
                    nc.vector.tensor_tensor(w_new[:], beta_ps[:],
                                            em_slice(i - 1), mult)
                    w = w_new

            # meet: Z = sum_t u_MEET[t] * beta_MEET[t] (ones-vector matmul);
            # the raw fp32 sums (~e^+-15 after the kappa prescale) go to the
            # host, which takes the log — keeps Ln and its ACT table load
            # off the device entirely.
            p = ufp.tile([T, BC], bf16, tag="meet")
            nc.vector.tensor_tensor(p[:], beta_ps[:], u[:], mult)
            srow = vp.tile([1, BC], fp32, tag="sum")
            nc.tensor.matmul(srow[:], ones_t[:], p[:], start=True, stop=True)
            dfin = sidep.tile([1, BC], fp32, tag="dfin")
            nc.vector.tensor_copy(dfin[:], srow[:])
            nc.sync.dma_start(denom[:], dfin[:])

    nc.compile()
    return nc


def _get_nc():
    global _nc_cache
    if _nc_cache is None:
        _nc_cache = _build_nc()
    return _nc_cache


def _ensure_ntff_hook_importable():
    """bass_utils imports antenv.axon_hooks when BASS_TRACE is set; this
    image's antenv package lacks that module, so provide a shim rather
    than crash (and enable profiling when the axon .so supports it)."""
    import sys
    import types
    try:
        import antenv.axon_hooks  # noqa: F401
        return
    except ImportError:
        pass
    try:
        import antenv
        from trn_agent_boot.trn_boot import _ntff_profile_via_ctypes
        hook = _ntff_profile_via_ctypes('/opt/axon/libaxon_pjrt.so')
    except Exception:
        try:
            import antenv
        except ImportError:
            return
        hook = None
    mod = types.ModuleType("antenv.axon_hooks")
    mod._hook = hook
    mod.get_axon_ntff_profile_hook = lambda: mod._hook
    mod.set_axon_ntff_profile_hook = lambda h: setattr(mod, "_hook", h)
    antenv.axon_hooks = mod
    sys.modules["antenv.axon_hooks"] = mod


def _kappa_host(em, trans, start):
    """Exact per-step log-mass growth of batch 0 (fp64 log-space forward)."""
    sc = start.astype(np.float64) + em[0, 0].astype(np.float64)
    t64 = trans.astype(np.float64)
    for i in range(1, em.shape[1]):
        x = sc[:, None] + t64 + em[0, i].astype(np.float64)[None, :]
        mx = x.max(axis=0)
        sc = mx + np.log(np.exp(x - mx[None, :]).sum(axis=0))
    mx = sc.max()
    return float((mx + np.log(np.exp(sc - mx).sum())) / (em.shape[1] - 1))


def _numerator_host(em, tags, mask, trans, start, end):
    em64 = em.astype(np.float64)
    tags = tags.astype(np.int64)
    bidx = np.arange(em.shape[0])
    score = start.astype(np.float64)[tags[:, 0]] + em64[bidx, 0, tags[:, 0]]
    trans_term = trans.astype(np.float64)[tags[:, 1:], tags[:, :-1]]
    em_term = np.take_along_axis(em64[:, 1:], tags[:, 1:, None], axis=2)[..., 0]
    m = mask[:, 1:].astype(np.float64)
    score = score + ((trans_term + em_term) * m).sum(axis=1)
    last_idx = mask.sum(axis=1).astype(np.int64) - 1
    last_tags = np.take_along_axis(tags, last_idx[:, None], axis=1)[:, 0]
    return score + end.astype(np.float64)[last_tags]


def _reference_host(em, tags, mask, trans, start, end):
    """Pure-numpy fp64 fallback (exact semantics incl. arbitrary masks)."""
    em64 = em.astype(np.float64)
    score = start.astype(np.float64) + em64[:, 0]  # [B, T]
    t64 = trans.astype(np.float64)
    for i in range(1, em.shape[1]):
        x = score[:, :, None] + t64[None] + em64[:, i][:, None, :]
        mx = x.max(axis=1)
        nxt = mx + np.log(np.exp(x - mx[:, None, :]).sum(axis=1))
        score = np.where(mask[:, i][:, None], nxt, score)
    x = score + end.astype(np.float64)
    mx = x.max(axis=1, keepdims=True)
    denom = (mx[:, 0] + np.log(np.exp(x - mx).sum(axis=1)))
    numer = _numerator_host(em, tags, mask, trans, start, end)
    return np.float32((denom - numer).mean())


def kernel(**inputs):
    global LAST_RESULTS
    em = np.asarray(inputs["emissions"], dtype=np.float32)
    tags = np.asarray(inputs["tags"])
    mask = np.asarray(inputs["mask"])
    trans = np.asarray(inputs["transitions"], dtype=np.float32)
    start = np.asarray(inputs["start_transitions"], dtype=np.float32)
    end = np.asarray(inputs["end_transitions"], dtype=np.float32)

    if not mask.all():
        # device scan assumes a dense mask (guaranteed by the input spec);
        # fall back to the exact host path otherwise
        return _reference_host(em, tags, mask, trans, start, end)

    _ensure_ntff_hook_importable()
    from concourse.bass_utils import run_bass_kernel_spmd

    nc = _get_nc()
    kap = _kappa_host(em, trans, start)
    bf = ml_dtypes.bfloat16
    a_exp_np = np.exp(trans).astype(bf)
    cpack_np = np.ascontiguousarray(
        np.concatenate([a_exp_np, np.ascontiguousarray(a_exp_np.T)], axis=1))
    se_np = np.stack([np.exp(start), np.exp(end)], axis=1).astype(np.float32)
    in_maps = []
    for cid in range(NCORES):
        emc = em[cid * BC:(cid + 1) * BC].copy()           # [BC, S, T]
        emc[:, 1:, :] -= np.float32(kap)
        em_t_np = np.ascontiguousarray(
            emc.astype(bf).transpose(2, 1, 0))             # [T, S, BC]
        in_maps.append({"em_t": em_t_np, "cpack": cpack_np, "se_exp": se_np})

    LAST_RESULTS = run_bass_kernel_spmd(nc, in_maps, list(range(NCORES)))
    zsums = np.concatenate(
        [LAST_RESULTS.results[cid]["denom"][0] for cid in range(NCORES)])

    if not (np.isfinite(zsums).all() and (zsums > 0).all()):
        return _reference_host(em, tags, mask, trans, start, end)
    denoms = np.log(zsums.astype(np.float64)) + (S - 1) * kap

    numer = _numerator_host(em, tags, mask, trans, start, end)
    return np.float32((denoms - numer).mean())



# revision 5
# speedup vs baseline: 2.7178x; 2.7178x over previous
"""CRF loss kernel for Trainium2 (8 NeuronCores, sequence-parallel).

reference: mean_b( logZ_b - score_b ) for a linear-chain CRF with
B=256, S=512, T=128.

The forward algorithm's 511-step recurrence u_s = (A^T u_{s-1}) o e_s
is latency-bound on device (~0.95 us per step-pair of matmul + DVE
multiply incl. PSUM drain/ack and semaphores), so instead of
data-parallel batch sharding this kernel shards the SEQUENCE: products
of positive matrices forget their starting direction at ~e^-1.9/step
(measured: L1 direction error 1.7e-7 after 8 steps), so each core
computes one ~64-step segment for ALL 256 batches, warm-starting its
incoming state with an 8-step warmup, and the host telescopes

    logZ_b = sum_c log r_cb - sum_{c>=1} log n_cb + 511*kappa

where r_c = 1^T M_seg_c v~_c (bilinear segment value, computed
meet-in-the-middle with an exact ones/end-vector backward chain) and
n_c = 1^T v~_c normalizes the warmup state at the segment boundary.
Serial depth per core: 36 rounds (vs 256 for batch-parallel
meet-in-the-middle), each round = fwd step + bwd step at width 256.

kappa (exact per-step log growth of batch 0, host fp64) is
pre-subtracted from the emissions so everything stays O(e^+-5) in bf16.
Emissions are pre-exponentiated on the host; segment-length differences
between cores (core 0 has no warmup, core 7 a 63-step segment) are
handled by per-core no-op pad steps: identity stationary slots in the
per-core cpack input plus all-ones emission tiles, so a single uniform
program runs on all 8 cores.

Numerator (score of the tagged path, ~0.1% of FLOPs) on host in fp64.
"""

import numpy as np
import ml_dtypes

B, S, T = 256, 512, 128
NCORES = 8
W = 8                      # warmup steps (direction error ~1.7e-7)
NF = 36                    # fwd slots per core
NB = 36                    # bwd slots per core
TAP = W                    # n_c tap after fwd slot index W (1-based slot 8)

_nc_cache = None
LAST_RESULTS = None


def _slot_plan(c):
    """Per-core (fwd_steps, bwd_steps) lists; None = pad (identity step).

    Segments: core 0: steps 1..64, cores 1..6: 64c+1..64c+64,
    core 7: 449..511 (63 steps).
    fwd covers [warmup] + left part, bwd covers right part reversed.
    Meet after fwd's last step.
    """
    if c == 0:
        f = list(range(1, 37))                       # 36 real, no warmup
        b = [None] * 8 + list(range(64, 36, -1))     # 8 pads + 64..37
    elif c < 7:
        f = list(range(64 * c - 7, 64 * c + 29))     # 8 warmup + 28 seg
        b = list(range(64 * c + 64, 64 * c + 28, -1))  # 64c+64..64c+29
    else:
        f = [None] + list(range(442, 477))           # 1 pad + 7 warmup + 28 seg
        b = [None] + list(range(511, 476, -1))       # 1 pad + 511..477
    assert len(f) == NF and len(b) == NB
    return f, b


def _build_nc():
    import concourse.bacc as bacc
    import concourse.mybir as mybir
    import concourse.tile as tile

    fp32 = mybir.dt.float32
    bf16 = mybir.dt.bfloat16
    mult = mybir.AluOpType.mult

    nc = bacc.Bacc("TRN2", target_bir_lowering=False, debug=False)

    emf = nc.dram_tensor("emf", [T, NF, B], bf16, kind="ExternalInput")
    emb = nc.dram_tensor("emb", [T, NB, B], bf16, kind="ExternalInput")
    finit = nc.dram_tensor("finit", [T, B], bf16, kind="ExternalInput")
    binit = nc.dram_tensor("binit", [T, B], bf16, kind="ExternalInput")
    # [F_X | A | B_X | B_Y | AT] -- per-core stationaries (Id for pad slots)
    cpack = nc.dram_tensor("cpack", [T, 5 * T], bf16, kind="ExternalInput")
    out = nc.dram_tensor("out", [2, B], fp32, kind="ExternalOutput")

    # emission chunks (slot ranges) sized small-to-large so both chains
    # start as soon as the first slot lands
    chunks = [(0, 1), (1, 2), (3, 4), (7, 6), (13, 8), (21, 8), (29, 7)]

    with tile.TileContext(nc) as tc:
        with (
            tc.tile_pool(name="const", bufs=1) as constp,
            tc.tile_pool(name="emfp", bufs=len(chunks)) as emfp,
            tc.tile_pool(name="embp", bufs=len(chunks)) as embp,
            tc.tile_pool(name="uf", bufs=2) as ufp,
            tc.tile_pool(name="wb", bufs=2) as wbp,
            tc.tile_pool(name="vps", bufs=2, space="PSUM") as vp,
            tc.tile_pool(name="bps", bufs=2, space="PSUM") as bp,
            tc.tile_pool(name="rps", bufs=2, space="PSUM") as rp,
            tc.tile_pool(name="side", bufs=1) as sidep,
        ):
            cp_tile = constp.tile([T, 5 * T], bf16)
            nc.sync.dma_start(cp_tile[:], cpack[:])
            fi_tile = constp.tile([T, B], bf16)
            nc.sync.dma_start(fi_tile[:], finit[:])
            bi_tile = constp.tile([T, B], bf16)
            nc.sync.dma_start(bi_tile[:], binit[:])

            fx_ap = cp_tile[:, 0:T]
            fa_ap = cp_tile[:, T:2 * T]
            bx_ap = cp_tile[:, 2 * T:3 * T]
            by_ap = cp_tile[:, 3 * T:4 * T]
            bat_ap = cp_tile[:, 4 * T:5 * T]

            ones_t = constp.tile([T, 1], bf16)
            nc.gpsimd.memset(ones_t[:], 1.0)
            out_r = sidep.tile([1, B], fp32)
            out_n = sidep.tile([1, B], fp32)

            emf_tiles = {}
            emb_tiles = {}

            def load_chunk(ci):
                s0, ln = chunks[ci]
                tf = emfp.tile([T, ln, B], bf16, tag="emf")
                nc.sync.dma_start(tf[:], emf[:, s0:s0 + ln, :])
                emf_tiles[ci] = tf
                tb = embp.tile([T, ln, B], bf16, tag="emb")
                nc.sync.dma_start(tb[:], emb[:, s0:s0 + ln, :])
                emb_tiles[ci] = tb

            def em_slice(tiles, k):
                for ci, (s0, ln) in enumerate(chunks):
                    if s0 <= k < s0 + ln:
                        return tiles[ci][:, k - s0, :]
                raise AssertionError(k)

            for ci in range(len(chunks)):
                load_chunk(ci)

            def fstat(k):
                return fx_ap if k == 0 else fa_ap

            def bstat(k):
                return bx_ap if k == 0 else (by_ap if k < 8 else bat_ap)

            u = fi_tile
            z_sb = bi_tile          # bwd state in SBUF only before slot 0
            zp = None               # bwd state in PSUM thereafter
            for k in range(NF):
                # fwd slot k: u <- (stat^T u) o e_f[k]
                vf = vp.tile([T, B], fp32, tag="vf")
                nc.tensor.matmul(vf[:], fstat(k), u[:], start=True, stop=True)
                u_new = ufp.tile([T, B], bf16)
                nc.vector.tensor_tensor(u_new[:], vf[:], em_slice(emf_tiles, k), mult)
                u = u_new
                # bwd slot k: z <- stat^T (e_b[k] o z)
                tmp = wbp.tile([T, B], bf16)
                zsrc = z_sb[:] if zp is None else zp[:]
                nc.vector.tensor_tensor(tmp[:], zsrc, em_slice(emb_tiles, k), mult)
                zp = bp.tile([T, B], fp32, tag="vb")
                nc.tensor.matmul(zp[:], bstat(k), tmp[:], start=True, stop=True)
                if k == TAP - 1:
                    # n_c = 1^T u after slot W (pre-segment boundary state)
                    nps = rp.tile([1, B], fp32, tag="nrow")
                    nc.tensor.matmul(nps[:], ones_t[:], u[:], start=True, stop=True)
                    nc.vector.tensor_copy(out_n[:], nps[:])

            # meet: r_c = sum_t z[t,b] * u[t,b]
            prod = ufp.tile([T, B], bf16, tag="meet")
            nc.vector.tensor_tensor(prod[:], zp[:], u[:], mult)
            rps_t = rp.tile([1, B], fp32, tag="rrow")
            nc.tensor.matmul(rps_t[:], ones_t[:], prod[:], start=True, stop=True)
            nc.vector.tensor_copy(out_r[:], rps_t[:])
            nc.sync.dma_start(out[0:1, :], out_r[:])
            nc.sync.dma_start(out[1:2, :], out_n[:])

    nc.compile()
    return nc


def _get_nc():
    global _nc_cache
    if _nc_cache is None:
        _nc_cache = _build_nc()
    return _nc_cache


def _ensure_ntff_hook_importable():
    """bass_utils imports antenv.axon_hooks when BASS_TRACE is set; this
    image's antenv package lacks that module, so provide a shim rather
    than crash (and enable profiling when the axon .so supports it)."""
    import sys
    import types
    try:
        import antenv.axon_hooks  # noqa: F401
        return
    except ImportError:
        pass
    try:
        import antenv
        from trn_agent_boot.trn_boot import _ntff_profile_via_ctypes
        hook = _ntff_profile_via_ctypes('/opt/axon/libaxon_pjrt.so')
    except Exception:
        try:
            import antenv
        except ImportError:
            return
        hook = None
    mod = types.ModuleType("antenv.axon_hooks")
    mod._hook = hook
    mod.get_axon_ntff_profile_hook = lambda: mod._hook
    mod.set_axon_ntff_profile_hook = lambda h: setattr(mod, "_hook", h)
    antenv.axon_hooks = mod
    sys.modules["antenv.axon_hooks"] = mod


def _kappa_host(em, trans, start):
    """Exact per-step log-mass growth of batch 0 (fp64 log-space forward)."""
    sc = start.astype(np.float64) + em[0, 0].astype(np.float64)
    t64 = trans.astype(np.float64)
    for i in range(1, em.shape[1]):
        x = sc[:, None] + t64 + em[0, i].astype(np.float64)[None, :]
        mx = x.max(axis=0)
        sc = mx + np.log(np.exp(x - mx[None, :]).sum(axis=0))
    mx = sc.max()
    return float((mx + np.log(np.exp(sc - mx).sum())) / (em.shape[1] - 1))


def _numerator_host(em, tags, mask, trans, start, end):
    em64 = em.astype(np.float64)
    tags = tags.astype(np.int64)
    bidx = np.arange(em.shape[0])
    score = start.astype(np.float64)[tags[:, 0]] + em64[bidx, 0, tags[:, 0]]
    trans_term = trans.astype(np.float64)[tags[:, 1:], tags[:, :-1]]
    em_term = np.take_along_axis(em64[:, 1:], tags[:, 1:, None], axis=2)[..., 0]
    m = mask[:, 1:].astype(np.float64)
    score = score + ((trans_term + em_term) * m).sum(axis=1)
    last_idx = mask.sum(axis=1).astype(np.int64) - 1
    last_tags = np.take_along_axis(tags, last_idx[:, None], axis=1)[:, 0]
    return score + end.astype(np.float64)[last_tags]


def _reference_host(em, tags, mask, trans, start, end):
    """Pure-numpy fp64 fallback (exact semantics incl. arbitrary masks)."""
    em64 = em.astype(np.float64)
    score = start.astype(np.float64) + em64[:, 0]  # [B, T]
    t64 = trans.astype(np.float64)
    for i in range(1, em.shape[1]):
        x = score[:, :, None] + t64[None] + em64[:, i][:, None, :]
        mx = x.max(axis=1)
        nxt = mx + np.log(np.exp(x - mx[:, None, :]).sum(axis=1))
        score = np.where(mask[:, i][:, None], nxt, score)
    x = score + end.astype(np.float64)
    mx = x.max(axis=1, keepdims=True)
    denom = (mx[:, 0] + np.log(np.exp(x - mx).sum(axis=1)))
    numer = _numerator_host(em, tags, mask, trans, start, end)
    return np.float32((denom - numer).mean())


def kernel(**inputs):
    global LAST_RESULTS
    em = np.asarray(inputs["emissions"], dtype=np.float32)
    tags = np.asarray(inputs["tags"])
    mask = np.asarray(inputs["mask"])
    trans = np.asarray(inputs["transitions"], dtype=np.float32)
    start = np.asarray(inputs["start_transitions"], dtype=np.float32)
    end = np.asarray(inputs["end_transitions"], dtype=np.float32)

    if not mask.all():
        # device scan assumes a dense mask (guaranteed by the input spec);
        # fall back to the exact host path otherwise
        return _reference_host(em, tags, mask, trans, start, end)

    _ensure_ntff_hook_importable()
    from concourse.bass_utils import run_bass_kernel_spmd

    nc = _get_nc()
    kap = _kappa_host(em, trans, start)
    bf = ml_dtypes.bfloat16

    # exp-space, kappa-scaled emission multipliers, [T, S, B]
    ex = np.exp(em.transpose(2, 1, 0) - np.float32(kap)).astype(bf)
    ones_tb = np.ones((T, B), dtype=bf)
    a_exp = np.exp(trans).astype(bf)
    at_exp = np.ascontiguousarray(a_exp.T)
    id_t = np.eye(T, dtype=bf)
    u0 = np.ascontiguousarray(
        np.exp(start[None, :] + em[:, 0, :]).T.astype(bf))      # [T, B]
    endv = np.ascontiguousarray(
        np.tile(np.exp(end).astype(bf)[:, None], (1, B)))

    def gather(steps):
        cols = [ones_tb[:, None, :] if s is None else ex[:, s:s + 1, :]
                for s in steps]
        return np.ascontiguousarray(np.concatenate(cols, axis=1))

    in_maps = []
    spans_r = np.zeros(NCORES)
    spans_n = np.zeros(NCORES)
    for c in range(NCORES):
        f, b = _slot_plan(c)
        spans_r[c] = sum(s is not None for s in f) + sum(s is not None for s in b)
        spans_n[c] = sum(s is not None for s in f[:TAP])
        fx = id_t if c == 7 else a_exp
        bx = id_t if c in (0, 7) else at_exp
        by = id_t if c == 0 else at_exp
        cpack_np = np.ascontiguousarray(
            np.concatenate([fx, a_exp, bx, by, at_exp], axis=1))
        in_maps.append({
            "emf": gather(f),
            "emb": gather(b),
            "finit": u0 if c == 0 else ones_tb,
            "binit": endv if c == 7 else ones_tb,
            "cpack": cpack_np,
        })

    LAST_RESULTS = run_bass_kernel_spmd(nc, in_maps, list(range(NCORES)))
    outs = np.stack([LAST_RESULTS.results[c]["out"] for c in range(NCORES)])
    r = outs[:, 0, :].astype(np.float64)    # [8, B]
    n = outs[:, 1, :].astype(np.float64)

    ok = np.isfinite(r).all() and (r > 0).all()
    ok = ok and np.isfinite(n[1:]).all() and (n[1:] > 0).all()
    if not ok:
        return _reference_host(em, tags, mask, trans, start, end)

    logz = (np.log(r).sum(axis=0) - np.log(n[1:]).sum(axis=0)
            + kap * (spans_r.sum() - spans_n[1:].sum()))
    numer = _numerator_host(em, tags, mask, trans, start, end)
    return np.float32((logz - numer).mean())


# revision 9
# speedup vs baseline: 2.7819x; 1.0236x over previous
"""CRF loss kernel for Trainium2 (8 NeuronCores, sequence-parallel).

reference: mean_b( logZ_b - score_b ) for a linear-chain CRF with
B=256, S=512, T=128.

The forward algorithm's 511-step recurrence u_s = (A^T u_{s-1}) o e_s
is latency-bound on device (~0.9 us per step of matmul + DVE multiply
incl. PSUM drain/ack and semaphores), so instead of data-parallel batch
sharding this kernel shards the SEQUENCE: products of positive matrices
forget their starting direction at ~e^-1.9/step (measured: L1 direction
error 2.9e-4 after 4 steps), so each core computes one ~64-step segment
for ALL 256 batches, warm-starting its incoming state with a 4-step
warmup, and the host telescopes

    logZ_b = sum_c log r_cb - sum_{c>=1} log n_cb + 511*kappa

where r_c = 1^T M_seg_c v~_c (bilinear segment value, computed
meet-in-the-middle with an exact ones/end-vector backward chain) and
n_c = 1^T v~_c normalizes the warmup state at the segment boundary.
Serial depth per core: 34 rounds (vs 256 for batch-parallel
meet-in-the-middle), each round = fwd step + bwd step at width 256,
anti-phased so TensorE and VectorE ping-pong between the two chains.

kappa (exact per-step log growth of batch 0, host fp64) is
pre-subtracted from the emissions so everything stays O(e^+-5) in bf16.
Emissions are pre-exponentiated on the host; segment-length differences
between cores (core 0 has no warmup, core 7 a 63-step segment) are
handled by per-core no-op pad steps: identity stationary slots in the
per-core boot inputs plus all-ones emission tiles, so a single uniform
program runs on all 8 cores. Emission feed uses two parallel DMA
trigger queues (Sync for fwd, GpSimd for bwd).

Numerator (score of the tagged path, ~0.1% of FLOPs) on host in fp64.
"""

import numpy as np
import ml_dtypes

B, S, T = 256, 512, 128
NCORES = 8
NF = NB = 34               # fwd/bwd slots per core
TAPK = 3                   # n_c tap after fwd slot index 3 (4 slots)

_nc_cache = None
LAST_RESULTS = None


def _slot_plan(c):
    """Per-core (fwd_steps, bwd_steps); None = pad (identity step).

    Segments: core 0: steps 1..64, cores 1..6: 64c+1..64c+64,
    core 7: 449..511. Warmup: 4 steps before the segment (3 + one pad
    for core 7; none for core 0). Meet after fwd's last step.
    """
    if c == 0:
        f = list(range(1, 35))                      # 34 real, no warmup
        b = [None] * 4 + list(range(64, 34, -1))    # 4 pads + 64..35
    elif c < 7:
        f = list(range(64 * c - 3, 64 * c + 31))    # 4 warm + 30 seg
        b = list(range(64 * c + 64, 64 * c + 30, -1))  # 34 real
    else:
        f = [None] + list(range(446, 479))          # pad + 3 warm + 30 seg
        b = [None] + list(range(511, 478, -1))      # pad + 511..479
    assert len(f) == NF and len(b) == NB
    return f, b


def _build_nc():
    import concourse.bacc as bacc
    import concourse.mybir as mybir
    import concourse.tile as tile

    fp32 = mybir.dt.float32
    bf16 = mybir.dt.bfloat16
    mult = mybir.AluOpType.mult

    nc = bacc.Bacc("TRN2", target_bir_lowering=False, debug=False)

    emf = nc.dram_tensor("emf", [T, NF, B], bf16, kind="ExternalInput")
    emb = nc.dram_tensor("emb", [T, NB, B], bf16, kind="ExternalInput")
    # bootf = [F_X | A | finit], bootb = [B_X | B_Y | binit]
    bootf = nc.dram_tensor("bootf", [T, 2 * T + B], bf16, kind="ExternalInput")
    bootb = nc.dram_tensor("bootb", [T, 2 * T + B], bf16, kind="ExternalInput")
    atp = nc.dram_tensor("atp", [T, T], bf16, kind="ExternalInput")
    out = nc.dram_tensor("out", [1, 2 * B], fp32, kind="ExternalOutput")

    chunks = [(0, 1), (1, 3), (4, 6), (10, 8), (18, 8), (26, 8)]

    with tile.TileContext(nc) as tc:
        with (
            tc.tile_pool(name="const", bufs=1) as constp,
            tc.tile_pool(name="emp", bufs=2 * len(chunks)) as emp,
            tc.tile_pool(name="sbp", bufs=4) as sbp,
            tc.tile_pool(name="vp", bufs=2, space="PSUM") as vp,
            tc.tile_pool(name="bp", bufs=2, space="PSUM") as bp,
            tc.tile_pool(name="rp", bufs=2, space="PSUM") as rp,
            tc.tile_pool(name="outp", bufs=1) as outp,
        ):
            bf_tile = constp.tile([T, 2 * T + B], bf16)
            nc.sync.dma_start(bf_tile[:], bootf[:])
            bb_tile = constp.tile([T, 2 * T + B], bf16)
            nc.gpsimd.dma_start(bb_tile[:], bootb[:])
            at_tile = constp.tile([T, T], bf16)
            nc.scalar.dma_start(at_tile[:], atp[:])

            fx_ap = bf_tile[:, 0:T]
            fa_ap = bf_tile[:, T:2 * T]
            fi_ap = bf_tile[:, 2 * T:2 * T + B]
            bx_ap = bb_tile[:, 0:T]
            by_ap = bb_tile[:, T:2 * T]
            bi_ap = bb_tile[:, 2 * T:2 * T + B]

            ones_t = constp.tile([T, 1], bf16)
            nc.gpsimd.memset(ones_t[:], 1.0)
            out_sb = outp.tile([1, 2 * B], fp32)

            emf_tiles = {}
            emb_tiles = {}

            def load_chunk(ci):
                s0, ln = chunks[ci]
                tf = emp.tile([T, ln, B], bf16, tag="emf")
                nc.sync.dma_start(tf[:], emf[:, s0:s0 + ln, :])
                emf_tiles[ci] = tf
                tb = emp.tile([T, ln, B], bf16, tag="emb")
                nc.gpsimd.dma_start(tb[:], emb[:, s0:s0 + ln, :])
                emb_tiles[ci] = tb

            def em_slice(tiles, k):
                for ci, (s0, ln) in enumerate(chunks):
                    if s0 <= k < s0 + ln:
                        return tiles[ci][:, k - s0, :]
                raise AssertionError(k)

            for ci in range(len(chunks)):
                load_chunk(ci)

            def fstat(k):
                return fx_ap if k == 0 else fa_ap

            def bstat(k):
                return bx_ap if k == 0 else (by_ap if k <= TAPK else at_tile[:])

            u = fi_ap
            z_prev = bi_ap          # bwd state (SBUF AP before slot 0)
            zp = None
            for k in range(NF):
                # fwd slot k: u <- (stat^T u) o e_f[k]
                vf = vp.tile([T, B], fp32, tag="vf")
                nc.tensor.matmul(vf[:], fstat(k), u, start=True, stop=True)
                u_new = sbp.tile([T, B], bf16, tag="u")
                nc.vector.tensor_tensor(u_new[:], vf[:], em_slice(emf_tiles, k), mult)
                u = u_new[:]
                # bwd slot k: z <- stat^T (e_b[k] o z)
                tmp = sbp.tile([T, B], bf16, tag="w")
                zsrc = z_prev if zp is None else zp[:]
                nc.vector.tensor_tensor(tmp[:], zsrc, em_slice(emb_tiles, k), mult)
                zp = bp.tile([T, B], fp32, tag="vb")
                nc.tensor.matmul(zp[:], bstat(k), tmp[:], start=True, stop=True)
                if k == TAPK:
                    # n_c = 1^T u after the warmup slots
                    nps = rp.tile([1, B], fp32, tag="nrow")
                    nc.tensor.matmul(nps[:], ones_t[:], u, start=True, stop=True)
                    nc.vector.tensor_copy(out_sb[0:1, B:2 * B], nps[:])

            # meet: r_c = sum_t z[t,b] * u[t,b]
            prod = sbp.tile([T, B], bf16, tag="u")
            nc.vector.tensor_tensor(prod[:], zp[:], u, mult)
            rps_t = rp.tile([1, B], fp32, tag="rrow")
            nc.tensor.matmul(rps_t[:], ones_t[:], prod[:], start=True, stop=True)
            nc.vector.tensor_copy(out_sb[0:1, 0:B], rps_t[:])
            nc.sync.dma_start(out[:], out_sb[:])

    nc.compile()
    return nc


def _get_nc():
    global _nc_cache
    if _nc_cache is None:
        _nc_cache = _build_nc()
    return _nc_cache


def _ensure_ntff_hook_importable():
    """bass_utils imports antenv.axon_hooks when BASS_TRACE is set; this
    image's antenv package lacks that module, so provide a shim rather
    than crash (and enable profiling when the axon .so supports it)."""
    import sys
    import types
    try:
        import antenv.axon_hooks  # noqa: F401
        return
    except ImportError:
        pass
    try:
        import antenv
        from trn_agent_boot.trn_boot import _ntff_profile_via_ctypes
        hook = _ntff_profile_via_ctypes('/opt/axon/libaxon_pjrt.so')
    except Exception:
        try:
            import antenv
        except ImportError:
            return
        hook = None
    mod = types.ModuleType("antenv.axon_hooks")
    mod._hook = hook
    mod.get_axon_ntff_profile_hook = lambda: mod._hook
    mod.set_axon_ntff_profile_hook = lambda h: setattr(mod, "_hook", h)
    antenv.axon_hooks = mod
    sys.modules["antenv.axon_hooks"] = mod


def _kappa_host(em, trans, start):
    """Exact per-step log-mass growth of batch 0 (fp64 log-space forward)."""
    sc = start.astype(np.float64) + em[0, 0].astype(np.float64)
    t64 = trans.astype(np.float64)
    for i in range(1, em.shape[1]):
        x = sc[:, None] + t64 + em[0, i].astype(np.float64)[None, :]
        mx = x.max(axis=0)
        sc = mx + np.log(np.exp(x - mx[None, :]).sum(axis=0))
    mx = sc.max()
    return float((mx + np.log(np.exp(sc - mx).sum())) / (em.shape[1] - 1))


def _numerator_host(em, tags, mask, trans, start, end):
    em64 = em.astype(np.float64)
    tags = tags.astype(np.int64)
    bidx = np.arange(em.shape[0])
    score = start.astype(np.float64)[tags[:, 0]] + em64[bidx, 0, tags[:, 0]]
    trans_term = trans.astype(np.float64)[tags[:, 1:], tags[:, :-1]]
    em_term = np.take_along_axis(em64[:, 1:], tags[:, 1:, None], axis=2)[..., 0]
    m = mask[:, 1:].astype(np.float64)
    score = score + ((trans_term + em_term) * m).sum(axis=1)
    last_idx = mask.sum(axis=1).astype(np.int64) - 1
    last_tags = np.take_along_axis(tags, last_idx[:, None], axis=1)[:, 0]
    return score + end.astype(np.float64)[last_tags]


def _reference_host(em, tags, mask, trans, start, end):
    """Pure-numpy fp64 fallback (exact semantics incl. arbitrary masks)."""
    em64 = em.astype(np.float64)
    score = start.astype(np.float64) + em64[:, 0]  # [B, T]
    t64 = trans.astype(np.float64)
    for i in range(1, em.shape[1]):
        x = score[:, :, None] + t64[None] + em64[:, i][:, None, :]
        mx = x.max(axis=1)
        nxt = mx + np.log(np.exp(x - mx[:, None, :]).sum(axis=1))
        score = np.where(mask[:, i][:, None], nxt, score)
    x = score + end.astype(np.float64)
    mx = x.max(axis=1, keepdims=True)
    denom = (mx[:, 0] + np.log(np.exp(x - mx).sum(axis=1)))
    numer = _numerator_host(em, tags, mask, trans, start, end)
    return np.float32((denom - numer).mean())


def kernel(**inputs):
    global LAST_RESULTS
    em = np.asarray(inputs["emissions"], dtype=np.float32)
    tags = np.asarray(inputs["tags"])
    mask = np.asarray(inputs["mask"])
    trans = np.asarray(inputs["transitions"], dtype=np.float32)
    start = np.asarray(inputs["start_transitions"], dtype=np.float32)
    end = np.asarray(inputs["end_transitions"], dtype=np.float32)

    if not mask.all():
        # device scan assumes a dense mask (guaranteed by the input spec);
        # fall back to the exact host path otherwise
        return _reference_host(em, tags, mask, trans, start, end)

    _ensure_ntff_hook_importable()
    from concourse.bass_utils import run_bass_kernel_spmd

    nc = _get_nc()
    kap = _kappa_host(em, trans, start)
    bf = ml_dtypes.bfloat16

    # exp-space, kappa-scaled emission multipliers, [T, S, B]
    ex = np.exp(em.transpose(2, 1, 0) - np.float32(kap)).astype(bf)
    ones_tb = np.ones((T, B), dtype=bf)
    a_exp = np.exp(trans).astype(bf)
    at_exp = np.ascontiguousarray(a_exp.T)
    id_t = np.eye(T, dtype=bf)
    u0 = np.ascontiguousarray(
        np.exp(start[None, :] + em[:, 0, :]).T.astype(bf))      # [T, B]
    endv = np.ascontiguousarray(
        np.tile(np.exp(end).astype(bf)[:, None], (1, B)))

    def gather(steps):
        cols = [ones_tb[:, None, :] if s is None else ex[:, s:s + 1, :]
                for s in steps]
        return np.ascontiguousarray(np.concatenate(cols, axis=1))

    in_maps = []
    spans_r = np.zeros(NCORES)
    spans_n = np.zeros(NCORES)
    for c in range(NCORES):
        f, b = _slot_plan(c)
        spans_r[c] = sum(s is not None for s in f) + sum(s is not None for s in b)
        spans_n[c] = sum(s is not None for s in f[:TAPK + 1])
        fx = id_t if c == 7 else a_exp
        bx = id_t if c in (0, 7) else at_exp
        by = id_t if c == 0 else at_exp
        in_maps.append({
            "emf": gather(f),
            "emb": gather(b),
            "bootf": np.ascontiguousarray(
                np.concatenate([fx, a_exp, u0 if c == 0 else ones_tb], axis=1)),
            "bootb": np.ascontiguousarray(
                np.concatenate([bx, by, endv if c == 7 else ones_tb], axis=1)),
            "atp": at_exp,
        })

    LAST_RESULTS = run_bass_kernel_spmd(nc, in_maps, list(range(NCORES)))
    outs = np.stack([LAST_RESULTS.results[c]["out"][0] for c in range(NCORES)])
    r = outs[:, :B].astype(np.float64)      # [8, B]
    n = outs[:, B:].astype(np.float64)

    ok = np.isfinite(r).all() and (r > 0).all()
    ok = ok and np.isfinite(n[1:]).all() and (n[1:] > 0).all()
    if not ok:
        return _reference_host(em, tags, mask, trans, start, end)

    logz = (np.log(r).sum(axis=0) - np.log(n[1:]).sum(axis=0)
            + kap * (spans_r.sum() - spans_n[1:].sum()))
    numer = _numerator_host(em, tags, mask, trans, start, end)
    return np.float32((logz - numer).mean())
